# revision 46
# baseline (speedup 1.0000x reference)
"""Trainium2 Bass kernel for nn_Attention_82781199663345 (sparse_attention).

Reference computation (see problem statement):
    q  = x @ Wq.T + bq                    -> heads interleaved: head n owns q[i*8+n]
    K  = (memory @ Wk.T + bk)             -> (L, H), same interleave
    QK[n,l] = (d**-.5) * sum_i q[i*8+n] * K[l, i*8+n]
    attn = softmax_l(QK)                  (pad-mask term is exactly 0.0 in fp32)
    V  = memory @ Wv.T + bv
    feat[n,i] = sum_l attn[n,l] * V[l, i*8+n]
    out = relu(concat(x, feat) @ Wo.T + bo)

Algebraic refactor used here (exact in real arithmetic):
  * QK[n,l] = memory[l] . w_n + c_n   with  w_n = sum_i q_s[i*8+n] * Wk[i*8+n, :]
    (c_n is constant per head -> cancels in softmax, dropped)
  * sum_l attn[n,l] = 1  =>  feat row n = (attn[n] @ memory) @ Wv.T + bv, sliced
    at columns i*8+n.
  So the only L-sized (memory-bound) work is:
      scores = memory @ W            (L, 8)
      ctx    = softmax(scores).T @ memory   (8, 2048)
  Everything else is O(H*MD) and done on host in fp32.

Device strategy (8 cores, sequence-parallel over L):
  Each core gets its 2048-row shard twice in fp8e4m3: natural (l,d) for the
  context pass and pre-transposed (d,l) for the scores pass (the PE contracts
  over the partition dim only).  Softmax uses no max-subtraction at all: the
  final ctx/s division cancels any constant factor, and scores for this
  operator are O(+-2.5) so exp(scores) is far from fp16 overflow.  The
  cross-core combine is then a pure sum: ctx = sum_c ctx_c, s = sum_c s_c.

v2 scheduling (from trace analysis of v1 at ~42-45us):
  * The Bass preamble's four Pool-engine const memsets cost ~3.4us because the
    walrus engine-entry rendezvous makes every engine wait for the slow Q7
    memsets.  Nothing in this kernel reads the const APs (exp bias is built on
    ACT), so the memsets are stripped along with the old preamble barrier.
  * The HBM stream is split across BOTH HWDGE rings (sync/SP and scalar/ACT):
    halves trigger-issue serialization and lets the stream start as soon as
    either engine clears its entry code.  wt/eye ride the scalar ring first so
    the sync ring's first trigger is already bulk memT.
  * softmax is pipelined per 512-l block: exp_j -> 4 PE transposes -> p_all
    copy_j -> pass-B matmuls for tiles 4j..4j+3.  Pass B starts after exp_0
    instead of after the whole softmax.
  * ctx accumulates into ONE psum bank (4 column-group matmul targets at
    partition offsets 0/32/64/96), so the drain is a single 128-partition DVE
    copy + a single output DMA instead of 4x 8-partition copies + 2 DMAs.
  * The last 4 memn tiles go as 256KiB singles split across the rings so the
    final completion receipt gates as little work as possible.
"""

import sys

import numpy as np

if "/opt/trn_rl_repo" not in sys.path:
    sys.path.insert(0, "/opt/trn_rl_repo")

H = 1024          # hidden dim
MD = 2048         # memory dim
L = 16384         # memory length
NH = 8            # heads
NCORES = 8
LSH = L // NCORES         # 2048 rows per core
DHEAD = H // NH           # 128
DC = MD // 128            # 16 contraction chunks (scores pass)
MEMT_FP8 = True           # scores-pass operand in fp8e4m3
MEMN_FP8 = True           # ctx-pass operand in fp8e4m3
LT = LSH // 128           # 16 l-tiles (context pass)
NB = 4                    # 512-wide psum column blocks

_CACHE = {}


def _build_nc():
    import concourse.bass as bass
    import concourse.mybir as mybir
    from concourse import tile

    fp16 = mybir.dt.float16
    fp8 = mybir.dt.float8e4
    f32 = mybir.dt.float32
    Exp = mybir.ActivationFunctionType.Exp
    memT_dt = fp8 if MEMT_FP8 else fp16
    memn_dt = fp8 if MEMN_FP8 else fp16

    nc = bass.Bass()
    # Bass.__init__ ends with four Pool-engine const memsets and an all-engine
    # barrier.  The walrus engine-entry rendezvous serializes every engine
    # behind the ~3.4us Q7 memsets, and nothing in this kernel consumes the
    # const APs (the exp bias is built on ACT), so drop the memsets AND the
    # barrier and let the DMA stream start immediately.
    preamble_strip = [
        i.name
        for f in nc.m.functions
        for b in f.blocks
        for i in b.instructions
        if isinstance(i, (mybir.InstDrain, mybir.InstEventSemaphore, mybir.InstMemset))
    ]
    # DRAM layouts are host-packed so every DMA reads a fully contiguous
    # 8 KiB run per partition (8 KiB descriptors instead of 2 KiB): memT is
    # 4 groups of 4 d-chunks, memn is 3 quads of 4 l-tiles + 4 tail singles.
    memT_d = nc.dram_tensor("memT", [2, 128, 8 * LSH], memT_dt,
                            kind="ExternalInput")
    memn8_d = nc.dram_tensor("memn8", [128, 8 * MD], memn_dt,
                             kind="ExternalInput")
    memn4_d = nc.dram_tensor("memn4", [128, 4 * MD], memn_dt,
                             kind="ExternalInput")
    memnl_d = nc.dram_tensor("memnl", [4, 128, MD], memn_dt,
                             kind="ExternalInput")
    wt_d = nc.dram_tensor("wt", [128, DC * NH], fp16, kind="ExternalInput")
    # Single output: 512 ctx columns + one column of per-(tile,head) softmax
    # partial sums (s), fp16.  One DMA with healthy 1 KiB descriptors — a
    # standalone (128,1) f32 s-output generated 4-byte descriptors whose
    # completion receipt straggled ~8 us.
    ctx_d = nc.dram_tensor("ctx", [128, 513], fp16, kind="ExternalOutput")
    # eye columns 0..7 feed the PE transposes; column 8 is all-ones and is
    # the moving operand of the s-reduction matmul (s = p_all.T @ ones).
    eye_np = np.zeros((128, NH + 1), dtype=np.float16)
    for j in range(4):
        eye_np[32 * j : 32 * j + NH, :NH] = np.eye(NH, dtype=np.float16)
    eye_np[:, NH] = 1.0
    eye_d = nc.inline_tensor(eye_np, "eye8")

    with tile.TileContext(nc) as tc:
        with (
            tc.tile_pool(name="const", bufs=1) as constp,
            tc.tile_pool(name="memTp", bufs=2) as memTp,
            tc.tile_pool(name="memnp", bufs=1) as memnp,
            tc.tile_pool(name="small", bufs=1) as smallp,
            tc.tile_pool(name="pssc", bufs=1, space=bass.MemorySpace.PSUM) as pssc,
            tc.tile_pool(name="psctx", bufs=1, space=bass.MemorySpace.PSUM) as psctx,
            tc.tile_pool(name="pstr", bufs=1, space=bass.MemorySpace.PSUM) as pstr,
        ):
            # Tiny pass-A operands ride the scalar (ACT) HWDGE ring so the
            # sync ring's FIFO leads with bulk memT.  Per-chunk 512 KiB DMAs
            # measure faster end-to-end than 2 MiB batches (big transfers
            # stall the DGE descriptor ring) and give fine-grained pipelining.
            wt_sb = constp.tile([128, DC * NH], fp16, tag="wt")
            nc.scalar.dma_start(out=wt_sb[:], in_=wt_d[:])
            eye_sb = constp.tile([128, NH + 1], fp16, tag="eye")
            nc.scalar.dma_start(out=eye_sb[:], in_=eye_d[:])

            # All bulk DMAs ride the sync ring ONLY: splitting across both
            # HWDGE rings measured ~8% slower (287 vs 324 GB/s) — the
            # interleaved packet streams lose HBM row locality.  1 MiB DMAs
            # with host-packed fully-contiguous 8 KiB-per-partition runs.
            memT_sb = []
            for g in range(2):
                t_ = memTp.tile([128, 8 * LSH], memT_dt, tag="memT")
                nc.sync.dma_start(out=t_[:], in_=memT_d[g])
                memT_sb.append(t_)

            def memT_chunk(c):
                return memT_sb[c // 8][:, (c % 8) * LSH : (c % 8 + 1) * LSH]

            # memn: one 2 MiB oct + one 1 MiB quad + 4 singles (256 KiB) at
            # the tail so the final completion receipts gate only one tile's
            # matmuls each.
            memn8_sb = memnp.tile([128, 8 * MD], memn_dt, tag="memn8")
            nc.sync.dma_start(out=memn8_sb[:], in_=memn8_d[:])
            memn4_sb = memnp.tile([128, 4 * MD], memn_dt, tag="memn4")
            nc.sync.dma_start(out=memn4_sb[:], in_=memn4_d[:])
            memn_last = []
            for i in range(4):
                t_ = memnp.tile([128, MD], memn_dt, tag="memnl", bufs=4)
                nc.sync.dma_start(out=t_[:], in_=memnl_d[i])
                memn_last.append(t_)

            def memn_tile(t):
                if t >= LT - 4:
                    return memn_last[t - (LT - 4)][:]
                if t < 8:
                    return memn8_sb[:, t * MD : (t + 1) * MD]
                return memn4_sb[:, (t - 8) * MD : (t - 7) * MD]

            # Pass A: scoresT[n, l] = sum_d w[d, n] * memT[d, l], accumulated
            # over 16 d-chunks (c outer so accumulation chases the DMA
            # arrivals).  All four 512-l column groups live in ONE psum bank
            # at partition offsets 0/32/64/96 — their accumulation groups are
            # disjoint partition ranges, and the serialized downstream
            # consumers (the ACT exps) read slices of the one tile anyway.
            sc_ps = pssc.tile([128, 512], f32, tag="sc")
            for c in range(DC):
                mt = memT_chunk(c)
                for nb in range(NB):
                    nc.tensor.matmul(
                        sc_ps[32 * nb : 32 * nb + NH, :],
                        wt_sb[:, c * NH : (c + 1) * NH],
                        mt[:, nb * 512 : (nb + 1) * 512],
                        start=(c == 0),
                        stop=(c == DC - 1),
                        tile_position=(0, 32 * nb),
                    )

            # The zero exp-bias is built on ACT itself (wt * 0.0 keeps it a
            # float immediate path) so nothing depends on the stripped
            # preamble memsets.
            zero_b = constp.tile([128, 1], f32, tag="zerob")
            nc.scalar.mul(zero_b[:], wt_sb[:, 0:1], 0.0)

            pT_sb = smallp.tile([128, 512], fp16, tag="pT")
            p_all = smallp.tile([128, LT * NH], fp16, tag="pall")
            tr_ps = [
                pstr.tile([128, 4 * NH], fp16, tag=f"tr{j}", name=f"tr{j}")
                for j in range(4)
            ]
            ctx_ps = psctx.tile([128, 512], f32, tag="ctx")

            # Softmax + pass B, pipelined per 512-l block j: as soon as block
            # j's exp lands, its four l-tiles are transposed (PE, col-packed
            # at 32-offsets), copied to SBUF, and their ctx matmuls issue.
            # exp_{j+1} runs on ACT underneath block j's matmuls, so pass B
            # starts ~one exp after pass A instead of after the whole softmax.
            # No max-subtraction: ctx/s cancels any constant factor and
            # scores are O(+-2.5), far from fp16 overflow.
            for j in range(4):
                nc.scalar.activation(
                    pT_sb[32 * j : 32 * j + NH, :],
                    sc_ps[32 * j : 32 * j + NH, :],
                    Exp, bias=zero_b[32 * j : 32 * j + NH, :],
                    scale=1.0,
                )
                for k in range(4):
                    t = 4 * j + k
                    nc.tensor.transpose(
                        tr_ps[j][:, k * NH : (k + 1) * NH],
                        pT_sb[32 * j : 32 * j + NH, k * 128 : (k + 1) * 128],
                        eye_sb[32 * j : 32 * j + NH, :NH],
                        tile_position=(32 * j, 0),
                    )
                nc.vector.tensor_copy(
                    p_all[:, j * 4 * NH : (j + 1) * 4 * NH], tr_ps[j][:]
                )
                for k in range(4):
                    t = 4 * j + k
                    for q in range(NB):
                        nc.tensor.matmul(
                            ctx_ps[32 * q : 32 * q + NH, :],
                            p_all[:, t * NH : (t + 1) * NH],
                            memn_tile(t)[:, q * 512 : (q + 1) * 512],
                            start=(t == 0),
                            stop=(t == LT - 1),
                            tile_position=(0, 32 * q),
                        )

            # s[(t,n)] = sum_l p_all[l, t*8+n] via one PE matmul against the
            # ones column of eye (the host sums the 16 per-tile partials per
            # head).  Cheaper than accum_out on the exps, which serializes a
            # ~350 ns accumulator readout after every exp.
            s_ps = pstr.tile([128, 1], f32, tag="sps", name="sps")
            nc.tensor.matmul(
                s_ps[:], p_all[:], eye_sb[:, NH : NH + 1], start=True, stop=True
            )

            # Drain ctx (128-partition DVE casts to fp16 — ctx is O(1e2) and
            # gets divided by s=O(1e4) on the host, so fp16's 2^-11 step is
            # ~1e-5 of the final feat scale) pipelined in column halves: the
            # first DMA's descriptor gen and packets overlap the second cast.
            # The s column copies early — it only needs the s matmul.
            ctx_sb = smallp.tile([128, 513], fp16, tag="ctxsb")
            nc.vector.tensor_copy(ctx_sb[:, 512:513], s_ps[:])
            nc.vector.tensor_copy(ctx_sb[:, 0:256], ctx_ps[:, 0:256])
            nc.sync.dma_start(out=ctx_d[:, 0:256], in_=ctx_sb[:, 0:256])
            nc.vector.tensor_copy(ctx_sb[:, 256:512], ctx_ps[:, 256:512])
            nc.scalar.dma_start(out=ctx_d[:, 256:513], in_=ctx_sb[:, 256:513])

    names = set(preamble_strip)
    for f in nc.m.functions:
        for b in f.blocks:
            insts = b.instructions
            keep = [i for i in insts if i.name not in names]
            if len(keep) != len(insts):
                insts[:] = keep

    _split_multiwait(nc, mybir)
    nc.finalize()
    return nc


def _split_multiwait(nc, mybir):
    """Split instructions carrying >1 semaphore wait into single-wait NoOps.

    The walrus build in this environment encodes exactly one sync wait per
    engine instruction (setupSyncWait raises "Too many sync wait commands"
    otherwise), but Tile attaches the full wait set of the kernel-tail drain
    to one instruction.  Hoist all but the last wait onto dedicated NoOps on
    the same engine queue, which preserves semantics exactly.
    """
    k = 0
    for func in nc.m.functions:
        for block in func.blocks:
            insts = block.instructions
            i = 0
            while i < len(insts):
                inst = insts[i]
                si = inst.sync_info
                if si is not None and si.on_wait and len(si.on_wait) > 1:
                    waits = list(si.on_wait)
                    nops = []
                    for w in waits[:-1]:
                        nop = mybir.InstNoOp(
                            name=f"I-waitsplit-{k}",
                            engine=inst.engine,
                            bass_nofuse=True,
                            sync_info=mybir.SyncInfo(on_wait=[w], on_update=[]),
                        )
                        k += 1
                        nc.register_instruction(nop)
                        nops.append(nop)
                    inst.sync_info = mybir.SyncInfo(
                        on_wait=[waits[-1]], on_update=list(si.on_update)
                    )
                    insts[i:i] = nops
                    i += len(nops)
                i += 1


def _get_nc():
    if "nc" not in _CACHE:
        _CACHE["nc"] = _build_nc()
    return _CACHE["nc"]


def _host_prep(inputs):
    x = np.asarray(inputs["x"], dtype=np.float32).reshape(-1)          # (1024,)
    memory = np.asarray(inputs["memory"], dtype=np.float32)            # (L, MD)
    Wq = np.asarray(inputs["Wq"], dtype=np.float32)
    bq = np.asarray(inputs["bq"], dtype=np.float32)
    Wk = np.asarray(inputs["Wk"], dtype=np.float32)

    q = (x @ Wq.T + bq) * (DHEAD ** -0.5)                              # (1024,)
    # w[:, n] = sum_i q[i*8+n] * Wk[i*8+n, :]
    wmat = np.einsum(
        "in,ind->dn", q.reshape(DHEAD, NH), Wk.reshape(DHEAD, NH, MD),
        optimize=True,
    ).astype(np.float32)                                               # (MD, 8)
    wt_packed = np.ascontiguousarray(
        wmat.reshape(DC, 128, NH).transpose(1, 0, 2).reshape(128, DC * NH)
    ).astype(np.float16)

    import ml_dtypes
    memT_np = ml_dtypes.float8_e4m3 if MEMT_FP8 else np.float16
    memn_np = ml_dtypes.float8_e4m3 if MEMN_FP8 else np.float16
    in_maps = []
    for c in range(NCORES):
        shard = memory[c * LSH : (c + 1) * LSH].astype(memn_np)        # (LSH, MD)
        shardT = memory[c * LSH : (c + 1) * LSH].T.astype(memT_np)     # (MD, LSH)
        # Partition-contiguous group packing: partition p holds the group's
        # chunk-rows back to back (4/8 KiB contiguous descriptors).
        memT_p = np.ascontiguousarray(
            shardT.reshape(2, 8, 128, LSH).transpose(0, 2, 1, 3)
            .reshape(2, 128, 8 * LSH)
        )
        memn8_p = np.ascontiguousarray(
            shard[: 8 * 128].reshape(8, 128, MD).transpose(1, 0, 2)
            .reshape(128, 8 * MD)
        )
        memn4_p = np.ascontiguousarray(
            shard[8 * 128 : 12 * 128].reshape(4, 128, MD).transpose(1, 0, 2)
            .reshape(128, 4 * MD)
        )
        memnl_p = np.ascontiguousarray(shard[12 * 128 :].reshape(4, 128, MD))
        in_maps.append(
            {
                "memT": memT_p,
                "memn8": memn8_p,
                "memn4": memn4_p,
                "memnl": memnl_p,
                "wt": wt_packed,
            }
        )
    return in_maps


def _host_finish(inputs, ctx_tot, s_tot):
    x = np.asarray(inputs["x"], dtype=np.float32).reshape(-1)
    Wv = np.asarray(inputs["Wv"], dtype=np.float32)
    bv = np.asarray(inputs["bv"], dtype=np.float32)
    Wo = np.asarray(inputs["Wo"], dtype=np.float32)
    bo = np.asarray(inputs["bo"], dtype=np.float32)

    ctx_norm = ctx_tot / s_tot                                         # (8, MD)
    feat_full = ctx_norm @ Wv.T + bv                                   # (8, 1024)
    feat = np.empty(H, dtype=np.float32)
    for n in range(NH):
        feat[n::NH] = feat_full[n, n::NH]
    ax = np.concatenate([x, feat])
    out = np.maximum(ax @ Wo.T + bo, 0.0).astype(np.float32)
    return out.reshape(1, 1, H)


def _run(inputs, trace=False, **spmd_kwargs):
    from concourse.bass_utils import run_bass_kernel_spmd

    nc = _get_nc()
    in_maps = _host_prep(inputs)
    res = run_bass_kernel_spmd(
        nc, in_maps, list(range(NCORES)), trace=trace, **spmd_kwargs
    )
    ctx_tot = np.zeros((NH, MD), dtype=np.float32)
    s_tot = np.zeros((NH, 1), dtype=np.float32)
    for r in res.results:
        out = r["ctx"].astype(np.float32)                              # (128, 513)
        # ctx layout: row 32q+n, col j (<512)  ->  ctx[n, 512q + j]
        c = out[:, :512].reshape(4, 32, 512)[:, :NH]
        ctx_tot += c.transpose(1, 0, 2).reshape(NH, MD)
        # s column: partition t*8+n holds tile t's partial sum for head n
        s_tot += out[:, 512].reshape(LT, NH).sum(axis=0)[:, None]
    return _host_finish(inputs, ctx_tot, s_tot), res


def kernel(**inputs) -> np.ndarray:
    out, _ = _run(inputs, trace=False)
    return out


# revision 47
# speedup vs baseline: 1.0962x; 1.0962x over previous
"""Trainium2 Bass kernel for nn_Attention_82781199663345 (sparse_attention).

Reference computation (see problem statement):
    q  = x @ Wq.T + bq                    -> heads interleaved: head n owns q[i*8+n]
    K  = (memory @ Wk.T + bk)             -> (L, H), same interleave
    QK[n,l] = (d**-.5) * sum_i q[i*8+n] * K[l, i*8+n]
    attn = softmax_l(QK)                  (pad-mask term is exactly 0.0 in fp32)
    V  = memory @ Wv.T + bv
    feat[n,i] = sum_l attn[n,l] * V[l, i*8+n]
    out = relu(concat(x, feat) @ Wo.T + bo)

Algebraic refactor used here (exact in real arithmetic):
  * QK[n,l] = memory[l] . w_n + c_n   with  w_n = sum_i q_s[i*8+n] * Wk[i*8+n, :]
    (c_n is constant per head -> cancels in softmax, dropped)
  * sum_l attn[n,l] = 1  =>  feat row n = (attn[n] @ memory) @ Wv.T + bv, sliced
    at columns i*8+n.
  So the only L-sized (memory-bound) work is:
      scores = memory @ W            (L, 8)
      ctx    = softmax(scores).T @ memory   (8, 2048)
  Everything else is O(H*MD) and done on host in fp32.

Device strategy (8 cores, sequence-parallel over L):
  Each core gets its 2048-row shard twice in fp8e4m3: natural (l,d) for the
  context pass and pre-transposed (d,l) for the scores pass (the PE contracts
  over the partition dim only).  Softmax uses no max-subtraction at all: the
  final ctx/s division cancels any constant factor, and scores for this
  operator are O(+-2.5) so exp(scores) is far from fp16 overflow.  The
  cross-core combine is then a pure sum: ctx = sum_c ctx_c, s = sum_c s_c.

v2 scheduling (from trace analysis of v1 at ~42-45us):
  * The Bass preamble's four Pool-engine const memsets cost ~3.4us because the
    walrus engine-entry rendezvous makes every engine wait for the slow Q7
    memsets.  Nothing in this kernel reads the const APs (exp bias is built on
    ACT), so the memsets are stripped along with the old preamble barrier.
  * The HBM stream is split across BOTH HWDGE rings (sync/SP and scalar/ACT):
    halves trigger-issue serialization and lets the stream start as soon as
    either engine clears its entry code.  wt/eye ride the scalar ring first so
    the sync ring's first trigger is already bulk memT.
  * softmax is pipelined per 512-l block: exp_j -> 4 PE transposes -> p_all
    copy_j -> pass-B matmuls for tiles 4j..4j+3.  Pass B starts after exp_0
    instead of after the whole softmax.
  * ctx accumulates into ONE psum bank (4 column-group matmul targets at
    partition offsets 0/32/64/96), so the drain is a single 128-partition DVE
    copy + a single output DMA instead of 4x 8-partition copies + 2 DMAs.
  * The last 4 memn tiles go as 256KiB singles split across the rings so the
    final completion receipt gates as little work as possible.
"""

import sys

import numpy as np

if "/opt/trn_rl_repo" not in sys.path:
    sys.path.insert(0, "/opt/trn_rl_repo")

H = 1024          # hidden dim
MD = 2048         # memory dim
L = 16384         # memory length
NH = 8            # heads
NCORES = 8
LSH = L // NCORES         # 2048 rows per core
DHEAD = H // NH           # 128
DC = MD // 128            # 16 contraction chunks (scores pass)
MEMT_FP8 = True           # scores-pass operand in fp8e4m3
MEMN_FP8 = True           # ctx-pass operand in fp8e4m3
LT = LSH // 128           # 16 l-tiles (context pass)
NB = 4                    # 512-wide psum column blocks

_CACHE = {}


def _build_nc():
    import concourse.bass as bass
    import concourse.mybir as mybir
    from concourse import tile

    fp16 = mybir.dt.float16
    fp8 = mybir.dt.float8e4
    f32 = mybir.dt.float32
    Exp = mybir.ActivationFunctionType.Exp
    memT_dt = fp8 if MEMT_FP8 else fp16
    memn_dt = fp8 if MEMN_FP8 else fp16

    nc = bass.Bass()
    # Bass.__init__ ends with four Pool-engine const memsets and an all-engine
    # barrier.  The walrus engine-entry rendezvous serializes every engine
    # behind the ~3.4us Q7 memsets, and nothing in this kernel consumes the
    # const APs (the exp bias is built on ACT), so drop the memsets AND the
    # barrier and let the DMA stream start immediately.
    preamble_strip = [
        i.name
        for f in nc.m.functions
        for b in f.blocks
        for i in b.instructions
        if isinstance(i, (mybir.InstDrain, mybir.InstEventSemaphore, mybir.InstMemset))
    ]
    # DRAM layouts are host-packed so every DMA reads a fully contiguous
    # 8 KiB run per partition (8 KiB descriptors instead of 2 KiB): memT is
    # 4 groups of 4 d-chunks, memn is 3 quads of 4 l-tiles + 4 tail singles.
    memT_d = nc.dram_tensor("memT", [2, 128, 8 * LSH], memT_dt,
                            kind="ExternalInput")
    memn8_d = nc.dram_tensor("memn8", [128, 8 * MD], memn_dt,
                             kind="ExternalInput")
    memn4_d = nc.dram_tensor("memn4", [128, 4 * MD], memn_dt,
                             kind="ExternalInput")
    memnl_d = nc.dram_tensor("memnl", [4, 128, MD], memn_dt,
                             kind="ExternalInput")
    wt_d = nc.dram_tensor("wt", [128, DC * NH], fp16, kind="ExternalInput")
    # Single output: 512 ctx columns + one column of per-(tile,head) softmax
    # partial sums (s), fp16.  One DMA with healthy 1 KiB descriptors — a
    # standalone (128,1) f32 s-output generated 4-byte descriptors whose
    # completion receipt straggled ~8 us.
    ctx_d = nc.dram_tensor("ctx", [128, 513], fp16, kind="ExternalOutput")
    # eye columns 0..7 feed the PE transposes; column 8 is all-ones and is
    # the moving operand of the s-reduction matmul (s = p_all.T @ ones).
    eye_np = np.zeros((128, NH + 1), dtype=np.float16)
    for j in range(4):
        eye_np[32 * j : 32 * j + NH, :NH] = np.eye(NH, dtype=np.float16)
    eye_np[:, NH] = 1.0
    eye_d = nc.inline_tensor(eye_np, "eye8")

    with tile.TileContext(nc) as tc:
        with (
            tc.tile_pool(name="const", bufs=1) as constp,
            tc.tile_pool(name="memTp", bufs=2) as memTp,
            tc.tile_pool(name="memnp", bufs=1) as memnp,
            tc.tile_pool(name="small", bufs=1) as smallp,
            tc.tile_pool(name="pssc", bufs=1, space=bass.MemorySpace.PSUM) as pssc,
            tc.tile_pool(name="psctx", bufs=1, space=bass.MemorySpace.PSUM) as psctx,
            tc.tile_pool(name="pstr", bufs=1, space=bass.MemorySpace.PSUM) as pstr,
        ):
            # Tiny pass-A operands ride the scalar (ACT) HWDGE ring so the
            # sync ring's FIFO leads with bulk memT.  Per-chunk 512 KiB DMAs
            # measure faster end-to-end than 2 MiB batches (big transfers
            # stall the DGE descriptor ring) and give fine-grained pipelining.
            wt_sb = constp.tile([128, DC * NH], fp16, tag="wt")
            nc.scalar.dma_start(out=wt_sb[:], in_=wt_d[:])
            eye_sb = constp.tile([128, NH + 1], fp16, tag="eye")
            nc.scalar.dma_start(out=eye_sb[:], in_=eye_d[:])

            # All bulk DMAs ride the sync ring ONLY: splitting across both
            # HWDGE rings measured ~8% slower (287 vs 324 GB/s) — the
            # interleaved packet streams lose HBM row locality.  1 MiB DMAs
            # with host-packed fully-contiguous 8 KiB-per-partition runs.
            memT_sb = []
            for g in range(2):
                t_ = memTp.tile([128, 8 * LSH], memT_dt, tag="memT")
                nc.sync.dma_start(out=t_[:], in_=memT_d[g])
                memT_sb.append(t_)

            def memT_chunk(c):
                return memT_sb[c // 8][:, (c % 8) * LSH : (c % 8 + 1) * LSH]

            # memn: one 2 MiB oct + one 1 MiB quad + 4 singles (256 KiB) at
            # the tail so the final completion receipts gate only one tile's
            # matmuls each.
            memn8_sb = memnp.tile([128, 8 * MD], memn_dt, tag="memn8")
            nc.sync.dma_start(out=memn8_sb[:], in_=memn8_d[:])
            memn4_sb = memnp.tile([128, 4 * MD], memn_dt, tag="memn4")
            nc.sync.dma_start(out=memn4_sb[:], in_=memn4_d[:])
            memn_last = []
            for i in range(4):
                t_ = memnp.tile([128, MD], memn_dt, tag="memnl", bufs=4)
                nc.sync.dma_start(out=t_[:], in_=memnl_d[i])
                memn_last.append(t_)

            def memn_tile(t):
                if t >= LT - 4:
                    return memn_last[t - (LT - 4)][:]
                if t < 8:
                    return memn8_sb[:, t * MD : (t + 1) * MD]
                return memn4_sb[:, (t - 8) * MD : (t - 7) * MD]

            # Pass A: scoresT[n, l] = sum_d w[d, n] * memT[d, l], accumulated
            # over 16 d-chunks (c outer so accumulation chases the DMA
            # arrivals).  All four 512-l column groups live in ONE psum bank
            # at partition offsets 0/32/64/96 — their accumulation groups are
            # disjoint partition ranges, and the serialized downstream
            # consumers (the ACT exps) read slices of the one tile anyway.
            sc_ps = pssc.tile([128, 512], f32, tag="sc")
            for c in range(DC):
                mt = memT_chunk(c)
                for nb in range(NB):
                    nc.tensor.matmul(
                        sc_ps[32 * nb : 32 * nb + NH, :],
                        wt_sb[:, c * NH : (c + 1) * NH],
                        mt[:, nb * 512 : (nb + 1) * 512],
                        start=(c == 0),
                        stop=(c == DC - 1),
                        tile_position=(0, 32 * nb),
                    )

            # The zero exp-bias is built on ACT itself (wt * 0.0 keeps it a
            # float immediate path) so nothing depends on the stripped
            # preamble memsets.
            zero_b = constp.tile([128, 1], f32, tag="zerob")
            nc.scalar.mul(zero_b[:], wt_sb[:, 0:1], 0.0)

            pT_sb = smallp.tile([128, 512], fp16, tag="pT")
            p_all = smallp.tile([128, LT * NH], fp16, tag="pall")
            tr_ps = [
                pstr.tile([128, 4 * NH], fp16, tag=f"tr{j}", name=f"tr{j}")
                for j in range(4)
            ]
            ctx_ps = psctx.tile([128, 512], f32, tag="ctx")

            # Softmax + pass B, pipelined per 512-l block j: as soon as block
            # j's exp lands, its four l-tiles are transposed (PE, col-packed
            # at 32-offsets), copied to SBUF, and their ctx matmuls issue.
            # exp_{j+1} runs on ACT underneath block j's matmuls, so pass B
            # starts ~one exp after pass A instead of after the whole softmax.
            # No max-subtraction: ctx/s cancels any constant factor and
            # scores are O(+-2.5), far from fp16 overflow.
            for j in range(4):
                nc.scalar.activation(
                    pT_sb[32 * j : 32 * j + NH, :],
                    sc_ps[32 * j : 32 * j + NH, :],
                    Exp, bias=zero_b[32 * j : 32 * j + NH, :],
                    scale=1.0,
                )
                for k in range(4):
                    t = 4 * j + k
                    nc.tensor.transpose(
                        tr_ps[j][:, k * NH : (k + 1) * NH],
                        pT_sb[32 * j : 32 * j + NH, k * 128 : (k + 1) * 128],
                        eye_sb[32 * j : 32 * j + NH, :NH],
                        tile_position=(32 * j, 0),
                    )
                nc.vector.tensor_copy(
                    p_all[:, j * 4 * NH : (j + 1) * 4 * NH], tr_ps[j][:]
                )
                for k in range(4):
                    t = 4 * j + k
                    for q in range(NB):
                        nc.tensor.matmul(
                            ctx_ps[32 * q : 32 * q + NH, :],
                            p_all[:, t * NH : (t + 1) * NH],
                            memn_tile(t)[:, q * 512 : (q + 1) * 512],
                            start=(t == 0),
                            stop=(t == LT - 1),
                            tile_position=(0, 32 * q),
                        )

            # s[(t,n)] = sum_l p_all[l, t*8+n] via one PE matmul against the
            # ones column of eye (the host sums the 16 per-tile partials per
            # head).  Cheaper than accum_out on the exps, which serializes a
            # ~350 ns accumulator readout after every exp.
            s_ps = pstr.tile([128, 1], f32, tag="sps", name="sps")
            nc.tensor.matmul(
                s_ps[:], p_all[:], eye_sb[:, NH : NH + 1], start=True, stop=True
            )

            # Drain ctx (128-partition casts to fp16 — ctx is O(1e2) and
            # gets divided by s=O(1e4) on the host, so fp16's 2^-11 step is
            # ~1e-5 of the final feat scale) in column halves on BOTH ACT
            # and DVE, each followed by its own DMA so descriptor gen and
            # packets overlap the other half's cast.  The s column copies
            # early on ACT — it only needs the s matmul.
            ctx_lo = smallp.tile([128, 256], fp16, tag="ctxlo")
            ctx_hi = smallp.tile([128, 257], fp16, tag="ctxhi")
            nc.scalar.copy(ctx_hi[:, 256:257], s_ps[:])
            nc.vector.tensor_copy(ctx_lo[:], ctx_ps[:, 0:256])
            nc.sync.dma_start(out=ctx_d[:, 0:256], in_=ctx_lo[:])
            nc.scalar.copy(ctx_hi[:, 0:256], ctx_ps[:, 256:512])
            nc.scalar.dma_start(out=ctx_d[:, 256:513], in_=ctx_hi[:])

    names = set(preamble_strip)
    for f in nc.m.functions:
        for b in f.blocks:
            insts = b.instructions
            keep = [i for i in insts if i.name not in names]
            if len(keep) != len(insts):
                insts[:] = keep

    _split_multiwait(nc, mybir)
    nc.finalize()
    return nc


def _split_multiwait(nc, mybir):
    """Split instructions carrying >1 semaphore wait into single-wait NoOps.

    The walrus build in this environment encodes exactly one sync wait per
    engine instruction (setupSyncWait raises "Too many sync wait commands"
    otherwise), but Tile attaches the full wait set of the kernel-tail drain
    to one instruction.  Hoist all but the last wait onto dedicated NoOps on
    the same engine queue, which preserves semantics exactly.
    """
    k = 0
    for func in nc.m.functions:
        for block in func.blocks:
            insts = block.instructions
            i = 0
            while i < len(insts):
                inst = insts[i]
                si = inst.sync_info
                if si is not None and si.on_wait and len(si.on_wait) > 1:
                    waits = list(si.on_wait)
                    nops = []
                    for w in waits[:-1]:
                        nop = mybir.InstNoOp(
                            name=f"I-waitsplit-{k}",
                            engine=inst.engine,
                            bass_nofuse=True,
                            sync_info=mybir.SyncInfo(on_wait=[w], on_update=[]),
                        )
                        k += 1
                        nc.register_instruction(nop)
                        nops.append(nop)
                    inst.sync_info = mybir.SyncInfo(
                        on_wait=[waits[-1]], on_update=list(si.on_update)
                    )
                    insts[i:i] = nops
                    i += len(nops)
                i += 1


def _get_nc():
    if "nc" not in _CACHE:
        _CACHE["nc"] = _build_nc()
    return _CACHE["nc"]


def _host_prep(inputs):
    x = np.asarray(inputs["x"], dtype=np.float32).reshape(-1)          # (1024,)
    memory = np.asarray(inputs["memory"], dtype=np.float32)            # (L, MD)
    Wq = np.asarray(inputs["Wq"], dtype=np.float32)
    bq = np.asarray(inputs["bq"], dtype=np.float32)
    Wk = np.asarray(inputs["Wk"], dtype=np.float32)

    q = (x @ Wq.T + bq) * (DHEAD ** -0.5)                              # (1024,)
    # w[:, n] = sum_i q[i*8+n] * Wk[i*8+n, :]
    wmat = np.einsum(
        "in,ind->dn", q.reshape(DHEAD, NH), Wk.reshape(DHEAD, NH, MD),
        optimize=True,
    ).astype(np.float32)                                               # (MD, 8)
    wt_packed = np.ascontiguousarray(
        wmat.reshape(DC, 128, NH).transpose(1, 0, 2).reshape(128, DC * NH)
    ).astype(np.float16)

    import ml_dtypes
    memT_np = ml_dtypes.float8_e4m3 if MEMT_FP8 else np.float16
    memn_np = ml_dtypes.float8_e4m3 if MEMN_FP8 else np.float16
    in_maps = []
    for c in range(NCORES):
        shard = memory[c * LSH : (c + 1) * LSH].astype(memn_np)        # (LSH, MD)
        shardT = memory[c * LSH : (c + 1) * LSH].T.astype(memT_np)     # (MD, LSH)
        # Partition-contiguous group packing: partition p holds the group's
        # chunk-rows back to back (4/8 KiB contiguous descriptors).
        memT_p = np.ascontiguousarray(
            shardT.reshape(2, 8, 128, LSH).transpose(0, 2, 1, 3)
            .reshape(2, 128, 8 * LSH)
        )
        memn8_p = np.ascontiguousarray(
            shard[: 8 * 128].reshape(8, 128, MD).transpose(1, 0, 2)
            .reshape(128, 8 * MD)
        )
        memn4_p = np.ascontiguousarray(
            shard[8 * 128 : 12 * 128].reshape(4, 128, MD).transpose(1, 0, 2)
            .reshape(128, 4 * MD)
        )
        memnl_p = np.ascontiguousarray(shard[12 * 128 :].reshape(4, 128, MD))
        in_maps.append(
            {
                "memT": memT_p,
                "memn8": memn8_p,
                "memn4": memn4_p,
                "memnl": memnl_p,
                "wt": wt_packed,
            }
        )
    return in_maps


def _host_finish(inputs, ctx_tot, s_tot):
    x = np.asarray(inputs["x"], dtype=np.float32).reshape(-1)
    Wv = np.asarray(inputs["Wv"], dtype=np.float32)
    bv = np.asarray(inputs["bv"], dtype=np.float32)
    Wo = np.asarray(inputs["Wo"], dtype=np.float32)
    bo = np.asarray(inputs["bo"], dtype=np.float32)

    ctx_norm = ctx_tot / s_tot                                         # (8, MD)
    feat_full = ctx_norm @ Wv.T + bv                                   # (8, 1024)
    feat = np.empty(H, dtype=np.float32)
    for n in range(NH):
        feat[n::NH] = feat_full[n, n::NH]
    ax = np.concatenate([x, feat])
    out = np.maximum(ax @ Wo.T + bo, 0.0).astype(np.float32)
    return out.reshape(1, 1, H)


def _run(inputs, trace=False, **spmd_kwargs):
    from concourse.bass_utils import run_bass_kernel_spmd

    nc = _get_nc()
    in_maps = _host_prep(inputs)
    res = run_bass_kernel_spmd(
        nc, in_maps, list(range(NCORES)), trace=trace, **spmd_kwargs
    )
    ctx_tot = np.zeros((NH, MD), dtype=np.float32)
    s_tot = np.zeros((NH, 1), dtype=np.float32)
    for r in res.results:
        out = r["ctx"].astype(np.float32)                              # (128, 513)
        # ctx layout: row 32q+n, col j (<512)  ->  ctx[n, 512q + j]
        c = out[:, :512].reshape(4, 32, 512)[:, :NH]
        ctx_tot += c.transpose(1, 0, 2).reshape(NH, MD)
        # s column: partition t*8+n holds tile t's partial sum for head n
        s_tot += out[:, 512].reshape(LT, NH).sum(axis=0)[:, None]
    return _host_finish(inputs, ctx_tot, s_tot), res


def kernel(**inputs) -> np.ndarray:
    out, _ = _run(inputs, trace=False)
    return out
